# revision 11
# baseline (speedup 1.0000x reference)
"""Trainium2 Bass kernel for nn_DSRA_Chunk_Layer (B=4,T=2048,D=1024,K=512,KR=16).

Sharding: 8 cores = (batch b, half h). Each core handles one batch's even (h=0)
or odd (h=1) 128-row t-blocks as queries ("my" blocks, permuted to the front),
with full-key causal attention balanced by the even/odd interleave. The
memory-bank path (top-16 read, novelty, write gate, V_agg) is t-sharded the
same way; partial S_next contributions combine linearly on the host.

Device program is identical on all cores (SPMD); all per-core differences are
pushed into the input data (permuted xT, per-core w weights, boundary masks).
"""
import os
import sys

sys.path.insert(0, "/opt/trn_rl_repo")
import numpy as np  # noqa: E402

B, T, D, K, KR = 4, 2048, 1024, 512, 16
ETA, LAM, EPS_COS = 0.1, 0.01, 1e-8
MY = T // 2          # rows per core
NB = MY // 128       # my t-blocks per core (8)
NEG = -1.0e30
N_CORES = 8

_compiled = None


def _build_nc():
    from concourse import bacc, tile, mybir
    import concourse.bass as bass

    F32 = mybir.dt.float32
    F32R = mybir.dt.float32r
    AF = mybir.ActivationFunctionType
    ALU = mybir.AluOpType
    AX = mybir.AxisListType
    ts = bass.ts

    nc = bacc.Bacc("TRN2", target_bir_lowering=False, debug=False,
                   num_devices=N_CORES)

    # ---- DRAM I/O ----
    xT_d = nc.dram_tensor("xT", [D + 1, T], F32R, kind="ExternalInput")
    wqT_d = nc.dram_tensor("wqT", [D + 1, D], F32R, kind="ExternalInput")
    wvT_d = nc.dram_tensor("wvT", [D + 1, D], F32R, kind="ExternalInput")
    sT_d = nc.dram_tensor("sT", [D, K], F32, kind="ExternalInput")
    smat_d = nc.dram_tensor("smat", [K, D], F32R, kind="ExternalInput")
    snT_d = nc.dram_tensor("snT", [D, K], F32R, kind="ExternalInput")
    wnT_d = nc.dram_tensor("wnT", [D + 2, K], F32R, kind="ExternalInput")
    wp_d = nc.dram_tensor("wp", [D + 1, 1], F32R, kind="ExternalInput")
    m2_d = nc.dram_tensor("m2", [D, D], F32R, kind="ExternalInput")
    wvec_d = nc.dram_tensor("wvec", [MY, 1], F32, kind="ExternalInput")
    dmask_d = nc.dram_tensor("dmask", [128, 128], F32, kind="ExternalInput")
    lmask_d = nc.dram_tensor("lmask", [128, 128], F32, kind="ExternalInput")
    ident_d = nc.dram_tensor("ident", [128, 128], F32, kind="ExternalInput")
    ones_d = nc.dram_tensor("ones1", [1, MY], F32R, kind="ExternalInput")

    out_d = nc.dram_tensor("out_half", [MY, D], F32, kind="ExternalOutput")
    spt_d = nc.dram_tensor("spartT", [D, K], F32, kind="ExternalOutput")

    with tile.TileContext(nc, pool_alloc_mode="queue") as tc:
        with (
            tc.tile_pool(name="persist", bufs=1) as pp,
            tc.tile_pool(name="dram", bufs=1, space="DRAM") as dp,
        ):
            qt_dram = dp.tile([8, 128, T], F32R)
            ctx_dram = dp.tile([8, 128, D], F32)
            p_dram = dp.tile([1, MY], F32)

            norms2 = pp.tile([128, NB], F32)
            nv_col = pp.tile([128, NB], F32)
            nvT = pp.tile([1, NB, 128], F32R)
            p_all = pp.tile([128, NB, 1], F32)
            w_col = pp.tile([128, NB, 1], F32)
            ident = pp.tile([128, 128], F32)
            identr = pp.tile([128, 128], F32R)
            ones_my = pp.tile([1, MY], F32R)
            nc.sync.dma_start(ident[:], ident_d[:])
            nc.sync.dma_start(identr[:], ident_d[:].bitcast(F32R))
            nc.sync.dma_start(w_col[:], wvec_d.rearrange("(i p) o -> p i o", p=128))
            nc.sync.dma_start(ones_my[:], ones_d[:])

            with tc.tile_pool(name="vall", bufs=1) as vap:
                v_all = vap.tile([128, 16, D], F32R)

                # ============ Phase A+B: projections (xT resident) =========
                with tc.tile_pool(name="xt", bufs=1) as xp:
                    xt = xp.tile([128, 8, T], F32R)
                    x_ones = xp.tile([1, T], F32R)
                    nc.sync.dma_start(xt[:], xT_d[0:D, :].rearrange("(c p) m -> p c m", p=128))
                    nc.sync.dma_start(x_ones[:], xT_d[D:D + 1, :])

                    def xt_lhs(c, sl):
                        return xt[:, c, sl] if c < 8 else x_ones[:, sl]

                    # --- Phase A: Q^T -> qt_dram, per feature block --------
                    with (
                        tc.tile_pool(name="wq", bufs=2) as wqp,
                        tc.tile_pool(name="qsb", bufs=2) as qsp,
                        tc.tile_pool(name="psA", bufs=4, space="PSUM") as psA,
                    ):
                        for co in range(8):
                            cs = ts(co, 128)
                            wq_sl = wqp.tile([128, 8, 128], F32R, tag="wq")
                            wq_b = wqp.tile([1, 128], F32R, tag="wqb")
                            nc.sync.dma_start(
                                wq_sl[:], wqT_d[0:D, cs].rearrange("(c p) m -> p c m", p=128))
                            nc.sync.dma_start(wq_b[:], wqT_d[D:D + 1, cs])
                            qsb = qsp.tile([128, T], F32R, tag="qsb")
                            pss = [psA.tile([128, 512], F32, tag="psA", name=f"psA{_s}") for _s in range(4)]
                            for c in range(9):
                                lhs = wq_sl[:, c, :] if c < 8 else wq_b[:]
                                for s in range(4):
                                    nc.tensor.matmul(pss[s][:], lhs, xt_lhs(c, ts(s, 512)),
                                                     start=(c == 0), stop=(c == 8))
                            for s in range(4):
                                nc.scalar.activation(qsb[:, ts(s, 512)], pss[s][:], AF.Copy)
                            nc.sync.dma_start(qt_dram[co], qsb[:])

                    # --- Phase B: V into v_all, norms2, p row --------------
                    with (
                        tc.tile_pool(name="wv", bufs=1) as wvp,
                        tc.tile_pool(name="sq", bufs=1) as sqp,
                        tc.tile_pool(name="psB", bufs=2, space="PSUM") as psB,
                        tc.tile_pool(name="psP", bufs=1, space="PSUM") as psP,
                    ):
                        wv = wvp.tile([128, 8, D], F32R)
                        wv_b = wvp.tile([1, D], F32R)
                        wp_sl = wvp.tile([128, 8, 1], F32R)
                        wp_b = wvp.tile([1, 1], F32R)
                        nc.sync.dma_start(wv[:], wvT_d[0:D, :].rearrange("(c p) m -> p c m", p=128))
                        nc.sync.dma_start(wv_b[:], wvT_d[D:D + 1, :])
                        nc.sync.dma_start(wp_sl[:], wp_d[0:D, :].rearrange("(c p) m -> p c m", p=128))
                        nc.sync.dma_start(wp_b[:], wp_d[D:D + 1, :])

                        sq = sqp.tile([128, D], F32)
                        for i in range(16):  # V rows -> v_all
                            ps = psB.tile([128, D], F32, tag="psB")
                            for c in range(9):
                                lhs = xt_lhs(c, ts(i, 128))
                                for s2 in range(D // 512):
                                    rhs = wv[:, c, ts(s2, 512)] if c < 8 else wv_b[:, ts(s2, 512)]
                                    nc.tensor.matmul(ps[:, ts(s2, 512)], lhs, rhs,
                                                     start=(c == 0), stop=(c == 8))
                            nc.scalar.activation(v_all[:, i, :], ps[:], AF.Copy)
                            if i < NB:
                                nc.scalar.activation(sq[:], v_all[:, i, :].bitcast(F32),
                                                     AF.Square, accum_out=norms2[:, i:i + 1])

                        # p row: sigmoid(x @ wp + b) for my columns
                        pp_ps = psP.tile([1, MY], F32)
                        for c in range(9):
                            lhs = wp_sl[:, c, :] if c < 8 else wp_b[:]
                            for s2 in range(MY // 512):
                                nc.tensor.matmul(pp_ps[:, ts(s2, 512)], lhs,
                                                 xt_lhs(c, ts(s2, 512)),
                                                 start=(c == 0), stop=(c == 8))
                        prow = sqp.tile([1, MY], F32)
                        nc.scalar.activation(prow[:], pp_ps[:], AF.Sigmoid)
                        nc.sync.dma_start(p_dram[:], prow[:])
                        nc.sync.dma_start(p_all[:], p_dram[:].rearrange("o (i p) -> p i o", p=128))

                # ============ Phases C+D (qt resident) =====================
                with tc.tile_pool(name="qt", bufs=1) as qtp:
                    qt = qtp.tile([128, 8, T], F32R)
                    nc.sync.dma_start(qt[:], qt_dram[:].rearrange("c p m -> p c m"))

                    # --- C1+C2+C3: read logits, top-16 softmax, context ----
                    with tc.tile_pool(name="rrT", bufs=1) as rrp:
                      rrT = rrp.tile([128, 4, MY], F32R)
                      with (
                        tc.tile_pool(name="st", bufs=1) as stp,
                        tc.tile_pool(name="c2", bufs=2) as c2p,
                        tc.tile_pool(name="psC", bufs=2, space="PSUM") as psC,
                        tc.tile_pool(name="psT", bufs=2, space="PSUM") as psTr,
                      ):
                        stile = stp.tile([128, 8, K], F32)
                        nc.sync.dma_start(stile[:], sT_d.rearrange("(c p) m -> p c m", p=128))
                        for i in range(NB):
                            isl = ts(i, 128)
                            ps = psC.tile([128, K], F32, tag="lg")
                            for c in range(8):
                                nc.tensor.matmul(ps[:], qt[:, c, isl].bitcast(F32),
                                                 stile[:, c, :], start=(c == 0), stop=(c == 7))
                            lg = c2p.tile([128, K], F32, tag="lg")
                            nc.scalar.activation(lg[:], ps[:], AF.Copy, scale=1.0 / 32.0)
                            m1 = c2p.tile([128, 8], F32, tag="m1")
                            m2t = c2p.tile([128, 8], F32, tag="m2")
                            t1 = c2p.tile([128, K], F32, tag="t1")
                            t2 = c2p.tile([128, K], F32, tag="t2")
                            nc.vector.max(m1[:], lg[:])
                            nc.vector.match_replace(t1[:], m1[:], lg[:], NEG)
                            nc.vector.max(m2t[:], t1[:])
                            nc.vector.match_replace(t2[:], m2t[:], t1[:], NEG)
                            nmax = c2p.tile([128, 1], F32, tag="nmax")
                            nc.vector.tensor_scalar_mul(nmax[:], m1[:, 0:1], -1.0)
                            e = c2p.tile([128, K], F32, tag="e")
                            nc.scalar.activation(e[:], lg[:], AF.Exp, bias=nmax[:, 0:1])
                            mk = c2p.tile([128, K], F32, tag="t1", name="mk")
                            nc.vector.tensor_scalar(mk[:], t2[:], NEG, None, ALU.is_equal)
                            den = c2p.tile([128, 1], F32, tag="den")
                            nc.vector.scalar_tensor_tensor(e[:], e[:], 1.0, mk[:],
                                                           ALU.mult, ALU.mult,
                                                           accum_out=den[:])
                            rden = c2p.tile([128, 1], F32, tag="rden")
                            nc.vector.reciprocal(rden[:], den[:])
                            rr = c2p.tile([128, K], F32, tag="t2", name="rr")
                            nc.vector.tensor_scalar_mul(rr[:], e[:], rden[:, 0:1])
                            for kc in range(4):
                                ptr = psTr.tile([128, 128], F32, tag="tr")
                                nc.tensor.transpose(ptr[:], rr[:, ts(kc, 128)], ident[:])
                                nc.vector.tensor_copy(rrT[:, kc, isl], ptr[:])
                      with (
                        tc.tile_pool(name="c3", bufs=2) as c3p,
                        tc.tile_pool(name="psC3", bufs=2, space="PSUM") as psC3,
                      ):
                        smt = c3p.tile([128, 4, D], F32R, tag="smt", bufs=1)
                        nc.sync.dma_start(smt[:], smat_d.rearrange("(c p) m -> p c m", p=128))
                        for i in range(NB):
                            ps = psC3.tile([128, D], F32, tag="ctx")
                            for kc in range(4):
                                for s2 in range(D // 512):
                                    nc.tensor.matmul(ps[:, ts(s2, 512)], rrT[:, kc, ts(i, 128)],
                                                     smt[:, kc, ts(s2, 512)],
                                                     start=(kc == 0), stop=(kc == 3))
                            cx = c3p.tile([128, D], F32, tag="cx")
                            nc.scalar.activation(cx[:], ps[:], AF.Copy)
                            nc.sync.dma_start(ctx_dram[i], cx[:])

                    # --- C4: novelty (vtm from PE transposes of v_all) -----
                    with (
                        tc.tile_pool(name="c4", bufs=1) as c4p,
                        tc.tile_pool(name="c4s", bufs=2) as c4s,
                        tc.tile_pool(name="psC4", bufs=2, space="PSUM") as psC4,
                        tc.tile_pool(name="psT4", bufs=2, space="PSUM") as psT4,
                    ):
                        vtm = c4p.tile([128, 8, MY], F32R)
                        snt = c4p.tile([128, 8, K], F32R)
                        nc.sync.dma_start(snt[:], snT_d.rearrange("(c p) m -> p c m", p=128))
                        for i in range(NB):
                            for c in range(8):
                                ptr = psT4.tile([128, 128], F32R, tag="vtr")
                                nc.tensor.transpose(ptr[:], v_all[:, i, ts(c, 128)], identr[:])
                                nc.vector.tensor_copy(vtm[:, c, ts(i, 128)], ptr[:])
                        for i in range(NB):
                            ps = psC4.tile([128, K], F32, tag="sim")
                            for c in range(8):
                                nc.tensor.matmul(ps[:], vtm[:, c, ts(i, 128)], snt[:, c, :],
                                                 start=(c == 0), stop=(c == 7))
                            nrm = c4s.tile([128, 1], F32, tag="nrm")
                            nc.scalar.activation(nrm[:], norms2[:, i:i + 1], AF.Sqrt)
                            ncl = c4s.tile([128, 1], F32, tag="ncl")
                            nc.vector.tensor_scalar_max(ncl[:], nrm[:], EPS_COS)
                            rin = c4s.tile([128, 1], F32, tag="rin")
                            nc.vector.reciprocal(rin[:], ncl[:])
                            sim = c4s.tile([128, K], F32, tag="sim")
                            nc.scalar.activation(sim[:], ps[:], AF.Copy, scale=rin[:, 0:1])
                            mx = c4s.tile([128, 1], F32, tag="mx")
                            nc.vector.tensor_reduce(mx[:], sim[:], axis=AX.X, op=ALU.max)
                            nc.scalar.activation(nv_col[:, i:i + 1], mx[:], AF.Copy,
                                                 bias=1.0, scale=-1.0)
                            ptr = psT4.tile([1, 128], F32, tag="ntr")
                            nc.tensor.transpose(ptr[:], nv_col[:, i:i + 1], ident[:])
                            nc.vector.tensor_copy(nvT[:, i, :], ptr[:])

                    # --- C5: r_write, V_aggT, S_partT ----------------------
                    with tc.tile_pool(name="vaTp", bufs=1) as vaTp:
                      vaT = vaTp.tile([128, 8, K], F32R)
                      with (
                        tc.tile_pool(name="c5", bufs=1) as c5p,
                        tc.tile_pool(name="c5s", bufs=2) as c5s,
                        tc.tile_pool(name="psC5", bufs=2, space="PSUM") as psC5,
                      ):
                        wn = c5p.tile([128, 8, K], F32R)
                        wn_nov = c5p.tile([1, K], F32R)
                        wn_b = c5p.tile([1, K], F32R)
                        nc.sync.dma_start(wn[:], wnT_d[0:D, :].rearrange("(c p) m -> p c m", p=128))
                        nc.sync.dma_start(wn_nov[:], wnT_d[D:D + 1, :])
                        nc.sync.dma_start(wn_b[:], wnT_d[D + 1:D + 2, :])
                        rww = c5p.tile([128, 8, K], F32R)
                        for i in range(NB):
                            isl = ts(i, 128)
                            ps = psC5.tile([128, K], F32, tag="rw")
                            for c in range(8):
                                nc.tensor.matmul(ps[:], qt[:, c, isl], wn[:, c, :],
                                                 start=(c == 0), stop=False)
                            nc.tensor.matmul(ps[:], nvT[:, i, :], wn_nov[:],
                                             start=False, stop=False)
                            nc.tensor.matmul(ps[:], ones_my[:, isl], wn_b[:],
                                             start=False, stop=True)
                            rw = c5s.tile([128, K], F32, tag="rw")
                            nc.scalar.activation(rw[:], ps[:], AF.Sigmoid)
                            nc.vector.tensor_scalar_mul(rww[:, i, :], rw[:],
                                                        w_col[:, i, 0:1])
                        for co in range(8):
                            ps = psC5.tile([128, K], F32, tag="va")
                            for i in range(NB):
                                nc.tensor.matmul(ps[:], v_all[:, i, ts(co, 128)],
                                                 rww[:, i, :],
                                                 start=(i == 0), stop=(i == NB - 1))
                            nc.vector.tensor_copy(vaT[:, co, :], ps[:])

                      with (
                        tc.tile_pool(name="c5b", bufs=1) as c5b,
                        tc.tile_pool(name="c5bs", bufs=2) as c5bs,
                        tc.tile_pool(name="psC5b", bufs=2, space="PSUM") as psC5b,
                      ):
                        m2sb = c5b.tile([128, 8, D], F32R)
                        nc.sync.dma_start(m2sb[:], m2_d.rearrange("(c p) m -> p c m", p=128))
                        for co in range(8):
                            ps = psC5b.tile([128, K], F32, tag="sp")
                            for c in range(8):
                                nc.tensor.matmul(ps[:], m2sb[:, c, ts(co, 128)],
                                                 vaT[:, c, :],
                                                 start=(c == 0), stop=(c == 7))
                            sp = c5bs.tile([128, K], F32, tag="sp")
                            nc.scalar.activation(sp[:], ps[:], AF.Copy)
                            nc.sync.dma_start(spt_d[ts(co, 128), :], sp[:])

                    # --- Phase D: causal attention + output mix ------------
                    with (
                        tc.tile_pool(name="att", bufs=2) as attp,
                        tc.tile_pool(name="attE", bufs=1) as attep,
                        tc.tile_pool(name="msk", bufs=1) as mskp,
                        tc.tile_pool(name="aT", bufs=3) as aTp,
                        tc.tile_pool(name="byp", bufs=2) as bypp,
                        tc.tile_pool(name="ctx", bufs=2) as ctxp,
                        tc.tile_pool(name="dsm", bufs=2) as dsmp,
                        tc.tile_pool(name="psS", bufs=2, space="PSUM") as psS,
                        tc.tile_pool(name="psO", bufs=2, space="PSUM") as psO,
                        tc.tile_pool(name="psT2", bufs=2, space="PSUM") as psT2,
                    ):
                        dmask = mskp.tile([128, 128], F32)
                        lmask = mskp.tile([128, 128], F32)
                        nc.sync.dma_start(dmask[:], dmask_d[:])
                        nc.sync.dma_start(lmask[:], lmask_d[:])

                        for j in range(NB):
                            jsl = ts(j, 128)
                            kw = (j + 1) * 128
                            att = attp.tile([128, T], F32, tag="att")
                            for reg in range(2):
                                base = reg * MY
                                for sub in range(0, kw, 512):
                                    w = min(512, kw - sub)
                                    ps = psS.tile([128, 512], F32, tag="sc")
                                    for c in range(8):
                                        nc.tensor.matmul(ps[:, 0:w], qt[:, c, jsl],
                                                         qt[:, c, base + sub:base + sub + w],
                                                         start=(c == 0), stop=(c == 7))
                                    nc.scalar.activation(att[:, base + sub:base + sub + w],
                                                         ps[:, 0:w], AF.Copy, scale=1.0 / 32.0)
                            nc.vector.tensor_add(att[:, jsl], att[:, jsl], dmask[:])
                            lsl = slice(MY + j * 128, MY + kw)
                            nc.vector.tensor_add(att[:, lsl], att[:, lsl], lmask[:])
                            mx1 = dsmp.tile([128, 1], F32, tag="mx1")
                            mx2 = dsmp.tile([128, 1], F32, tag="mx2")
                            nc.vector.tensor_reduce(mx1[:], att[:, 0:kw], axis=AX.X, op=ALU.max)
                            nc.vector.tensor_reduce(mx2[:], att[:, MY:MY + kw], axis=AX.X, op=ALU.max)
                            nmx = dsmp.tile([128, 1], F32, tag="nmx")
                            nc.vector.tensor_tensor(nmx[:], mx1[:], mx2[:], op=ALU.max)
                            nc.vector.tensor_scalar_mul(nmx[:], nmx[:], -1.0)
                            atE = attep.tile([128, T], F32, tag="atE")
                            rs1 = dsmp.tile([128, 1], F32, tag="rs1")
                            rs2 = dsmp.tile([128, 1], F32, tag="rs2")
                            nc.scalar.activation(atE[:, 0:kw], att[:, 0:kw], AF.Exp,
                                                 bias=nmx[:, 0:1], accum_out=rs1[:])
                            nc.scalar.activation(atE[:, MY:MY + kw], att[:, MY:MY + kw],
                                                 AF.Exp, bias=nmx[:, 0:1], accum_out=rs2[:])
                            den = dsmp.tile([128, 1], F32, tag="den")
                            nc.vector.tensor_add(den[:], rs1[:], rs2[:])
                            rinv = dsmp.tile([128, 1], F32, tag="rinv")
                            nc.vector.reciprocal(rinv[:], den[:])
                            ob = psO.tile([128, D], F32, tag="ob")
                            nvis = 2 * (j + 1)
                            for kb in range(nvis):
                                if kb <= j:
                                    lofs, vidx = kb * 128, kb
                                else:
                                    lofs, vidx = MY + (kb - (j + 1)) * 128, 8 + (kb - (j + 1))
                                ptr = psT2.tile([128, 128], F32, tag="atr")
                                nc.tensor.transpose(ptr[:], atE[:, lofs:lofs + 128], ident[:])
                                aT = aTp.tile([128, 128], F32R, tag="aT")
                                nc.vector.tensor_copy(aT[:], ptr[:])
                                for s2 in range(2):
                                    nc.tensor.matmul(ob[:, ts(s2, 512)], aT[:],
                                                     v_all[:, vidx, ts(s2, 512)],
                                                     start=(kb == 0), stop=(kb == nvis - 1))
                            byp = bypp.tile([128, D], F32, tag="byp")
                            nc.scalar.activation(byp[:, 0:512], ob[:, 0:512], AF.Copy,
                                                 scale=rinv[:, 0:1])
                            nc.scalar.activation(byp[:, 512:D], ob[:, 512:D], AF.Copy,
                                                 scale=rinv[:, 0:1])
                            ctx = ctxp.tile([128, D], F32, tag="ctx")
                            nc.sync.dma_start(ctx[:], ctx_dram[j])
                            nc.vector.tensor_sub(byp[:], byp[:], ctx[:])
                            nc.vector.scalar_tensor_tensor(byp[:], byp[:], p_all[:, j, 0:1],
                                                           ctx[:], ALU.mult, ALU.add)
                            nc.sync.dma_start(out_d[jsl, :], byp[:])

    nc.compile()
    return nc


def _host_prep(x, S_init, Wq_w, Wq_b, Wv_w, Wv_b, Wn_w, Wn_b, Wm_w, Wm_b):
    """Build the 8 per-core input maps."""
    f32 = np.float32
    S64 = S_init.astype(np.float64)
    S_cov = S64 @ S64.T + 1e-5 * np.eye(K)
    M = S64.T @ np.linalg.inv(S_cov) @ S64
    m2 = (ETA * (np.eye(D) - M)).astype(f32)

    sn = S64 / np.maximum(np.linalg.norm(S64, axis=1, keepdims=True), EPS_COS)
    snT = np.ascontiguousarray(sn.T.astype(f32))
    sT = np.ascontiguousarray(S_init.T.astype(f32))
    smat = np.ascontiguousarray(S_init.astype(f32))

    wqT = np.concatenate([Wq_w.T, Wq_b[None, :]], axis=0).astype(f32)
    wvT = np.concatenate([Wv_w.T, Wv_b[None, :]], axis=0).astype(f32)
    wnT = np.concatenate([Wn_w[:, 1:].T, Wn_w[:, 0][None, :], Wn_b[None, :]],
                         axis=0).astype(f32)
    # p gate folded through Wq: p_logit = x@(Wq^T wm) + (bq.wm + bm)
    wm64 = Wm_w[0].astype(np.float64)
    wp = np.concatenate([Wq_w.T.astype(np.float64) @ wm64,
                         [Wq_b.astype(np.float64) @ wm64 + float(Wm_b[0])]])
    wp = np.ascontiguousarray(wp[:, None].astype(f32))

    wfull = (1.0 - LAM) ** np.arange(T - 1, -1, -1, dtype=np.float64)
    wfull = (wfull / wfull.sum()).astype(f32)

    q = np.arange(128)[:, None]
    k = np.arange(128)[None, :]
    dmask = np.where(q >= k, 0.0, NEG).astype(f32)
    ident = np.eye(128, dtype=f32)

    in_maps = []
    perms = []
    for core in range(N_CORES):
        b, h = core // 2, core % 2
        my_blocks = list(range(h, 16, 2))
        oth_blocks = list(range(1 - h, 16, 2))
        perm = np.concatenate([np.arange(g * 128, (g + 1) * 128)
                               for g in my_blocks + oth_blocks])
        perms.append(perm)
        xp = x[b][perm]  # [T, D]
        xT = np.concatenate([np.ascontiguousarray(xp.T),
                             np.ones((1, T), f32)], axis=0)
        lmask = np.full((128, 128), NEG if h == 0 else 0.0, f32)
        in_maps.append({
            "xT": np.ascontiguousarray(xT.astype(f32)),
            "wqT": wqT, "wvT": wvT, "sT": sT, "smat": smat, "snT": snT,
            "wnT": wnT, "wp": wp, "m2": m2,
            "wvec": np.ascontiguousarray(wfull[perm[:MY]][:, None]),
            "dmask": dmask, "lmask": lmask, "ident": ident,
            "ones1": np.ones((1, MY), f32),
        })
    return in_maps, perms


def _gather(results, S_init):
    f32 = np.float32
    out = np.empty((B, T, D), f32)
    S_next = np.empty((B, K, D), f32)
    for b in range(B):
        for h in range(2):
            oh = results[2 * b + h]["out_half"]
            for j in range(NB):
                g = 2 * j + h
                out[b, g * 128:(g + 1) * 128] = oh[j * 128:(j + 1) * 128]
        S_next[b] = (f32(1.0 - LAM) * S_init
                     + results[2 * b]["spartT"].T
                     + results[2 * b + 1]["spartT"].T)
    return out, S_next


def kernel(x, S_init, Wq_w, Wq_b, Wv_w, Wv_b, Wn_w, Wn_b, Wm_w, Wm_b):
    global _compiled
    from concourse import bass_utils

    x = np.asarray(x, np.float32)
    S_init = np.asarray(S_init, np.float32)
    args = [np.asarray(a, np.float32) for a in
            (Wq_w, Wq_b, Wv_w, Wv_b, Wn_w, Wn_b, Wm_w, Wm_b)]

    if _compiled is None:
        _compiled = _build_nc()
    nc = _compiled

    in_maps, _ = _host_prep(x, S_init, *args)

    trace = bool(int(os.environ.get("BASS_KERNEL_TRACE", "0")))
    if trace:
        sys.path.insert(0, os.path.dirname(os.path.abspath(__file__)))
        import ntff_shim
        ntff_shim.install()
    res = bass_utils.run_bass_kernel_spmd(
        nc, in_maps, core_ids=list(range(N_CORES)), trace=trace)
    kernel.last_result = res
    return _gather(res.results, S_init)


# revision 13
# speedup vs baseline: 1.0432x; 1.0432x over previous
"""Trainium2 Bass kernel for nn_DSRA_Chunk_Layer (B=4,T=2048,D=1024,K=512,KR=16).

Sharding: 8 cores = (batch b, half h). Each core handles one batch's even (h=0)
or odd (h=1) 128-row t-blocks as queries ("my" blocks, permuted to the front),
with full-key causal attention balanced by the even/odd interleave. The
memory-bank path (top-16 read, novelty, write gate, V_agg) is t-sharded the
same way; partial S_next contributions combine linearly on the host.

Device program is identical on all cores (SPMD); all per-core differences are
pushed into the input data (permuted xT, per-core w weights, boundary masks).
"""
import os
import sys

sys.path.insert(0, "/opt/trn_rl_repo")
import numpy as np  # noqa: E402

B, T, D, K, KR = 4, 2048, 1024, 512, 16
ETA, LAM, EPS_COS = 0.1, 0.01, 1e-8
MY = T // 2          # rows per core
NB = MY // 128       # my t-blocks per core (8)
NEG = -1.0e30
N_CORES = 8

_compiled = None


def _build_nc():
    from concourse import bacc, tile, mybir
    import concourse.bass as bass

    F32 = mybir.dt.float32
    F32R = mybir.dt.float32r
    AF = mybir.ActivationFunctionType
    ALU = mybir.AluOpType
    AX = mybir.AxisListType
    ts = bass.ts

    nc = bacc.Bacc("TRN2", target_bir_lowering=False, debug=False,
                   num_devices=N_CORES)

    # ---- DRAM I/O ----
    xT_d = nc.dram_tensor("xT", [D + 1, T], F32R, kind="ExternalInput")
    wqT_d = nc.dram_tensor("wqT", [D + 1, D], F32R, kind="ExternalInput")
    wvT_d = nc.dram_tensor("wvT", [D + 1, D], F32R, kind="ExternalInput")
    sT_d = nc.dram_tensor("sT", [D, K], F32, kind="ExternalInput")
    smat_d = nc.dram_tensor("smat", [K, D], F32R, kind="ExternalInput")
    snT_d = nc.dram_tensor("snT", [D, K], F32R, kind="ExternalInput")
    wnT_d = nc.dram_tensor("wnT", [D + 2, K], F32R, kind="ExternalInput")
    wp_d = nc.dram_tensor("wp", [D + 1, 1], F32R, kind="ExternalInput")
    m2_d = nc.dram_tensor("m2", [D, D], F32R, kind="ExternalInput")
    wvec_d = nc.dram_tensor("wvec", [MY, 1], F32, kind="ExternalInput")
    dmask_d = nc.dram_tensor("dmask", [128, 128], F32, kind="ExternalInput")
    lmask_d = nc.dram_tensor("lmask", [128, 128], F32, kind="ExternalInput")
    ident_d = nc.dram_tensor("ident", [128, 128], F32, kind="ExternalInput")
    ones_d = nc.dram_tensor("ones1", [1, MY], F32R, kind="ExternalInput")

    out_d = nc.dram_tensor("out_half", [MY, D], F32, kind="ExternalOutput")
    spt_d = nc.dram_tensor("spartT", [D, K], F32, kind="ExternalOutput")

    with tile.TileContext(nc, pool_alloc_mode="queue") as tc:
        with (
            tc.tile_pool(name="persist", bufs=1) as pp,
            tc.tile_pool(name="dram", bufs=1, space="DRAM") as dp,
        ):
            qt_dram = dp.tile([8, 128, T], F32R)
            ctx_dram = dp.tile([8, 128, D], F32)
            p_dram = dp.tile([1, MY], F32)

            norms2 = pp.tile([128, NB], F32)
            nv_col = pp.tile([128, NB], F32)
            nvT = pp.tile([1, NB, 128], F32R)
            p_all = pp.tile([128, NB, 1], F32)
            w_col = pp.tile([128, NB, 1], F32)
            ident = pp.tile([128, 128], F32)
            identr = pp.tile([128, 128], F32R)
            nc.sync.dma_start(ident[:], ident_d[:])
            nc.sync.dma_start(identr[:], ident_d[:].bitcast(F32R))
            nc.sync.dma_start(w_col[:], wvec_d.rearrange("(i p) o -> p i o", p=128))

            with tc.tile_pool(name="vall", bufs=1) as vap:
                v_all = vap.tile([128, 16, D], F32R)

                # ============ Phase A+B: projections (xT resident) =========
                with tc.tile_pool(name="xt", bufs=1) as xp:
                    xt = xp.tile([128, 8, T], F32R)
                    x_ones = xp.tile([1, T], F32R)
                    nc.sync.dma_start(xt[:], xT_d[0:D, :].rearrange("(c p) m -> p c m", p=128))
                    nc.sync.dma_start(x_ones[:], xT_d[D:D + 1, :])

                    def xt_lhs(c, sl):
                        return xt[:, c, sl] if c < 8 else x_ones[:, sl]

                    # --- Phase A: Q^T -> qt_dram, per feature block --------
                    with (
                        tc.tile_pool(name="wq", bufs=2) as wqp,
                        tc.tile_pool(name="qsb", bufs=2) as qsp,
                        tc.tile_pool(name="psA", bufs=4, space="PSUM") as psA,
                    ):
                        for co in range(8):
                            cs = ts(co, 128)
                            wq_sl = wqp.tile([128, 8, 128], F32R, tag="wq")
                            wq_b = wqp.tile([1, 128], F32R, tag="wqb")
                            nc.sync.dma_start(
                                wq_sl[:], wqT_d[0:D, cs].rearrange("(c p) m -> p c m", p=128))
                            nc.sync.dma_start(wq_b[:], wqT_d[D:D + 1, cs])
                            qsb = qsp.tile([128, T], F32R, tag="qsb")
                            pss = [psA.tile([128, 512], F32, tag="psA", name=f"psA{_s}") for _s in range(4)]
                            for c in range(9):
                                lhs = wq_sl[:, c, :] if c < 8 else wq_b[:]
                                for s in range(4):
                                    nc.tensor.matmul(pss[s][:], lhs, xt_lhs(c, ts(s, 512)),
                                                     start=(c == 0), stop=(c == 8))
                            for s in range(4):
                                nc.scalar.activation(qsb[:, ts(s, 512)], pss[s][:], AF.Copy)
                            nc.sync.dma_start(qt_dram[co], qsb[:])

                    # --- Phase B: V into v_all, norms2, p row --------------
                    with (
                        tc.tile_pool(name="wv", bufs=1) as wvp,
                        tc.tile_pool(name="sq", bufs=1) as sqp,
                        tc.tile_pool(name="psB", bufs=2, space="PSUM") as psB,
                        tc.tile_pool(name="psP", bufs=1, space="PSUM") as psP,
                    ):
                        wv = wvp.tile([128, 8, D], F32R)
                        wv_b = wvp.tile([1, D], F32R)
                        wp_sl = wvp.tile([128, 8, 1], F32R)
                        wp_b = wvp.tile([1, 1], F32R)
                        nc.sync.dma_start(wv[:], wvT_d[0:D, :].rearrange("(c p) m -> p c m", p=128))
                        nc.sync.dma_start(wv_b[:], wvT_d[D:D + 1, :])
                        nc.sync.dma_start(wp_sl[:], wp_d[0:D, :].rearrange("(c p) m -> p c m", p=128))
                        nc.sync.dma_start(wp_b[:], wp_d[D:D + 1, :])

                        for i in range(16):  # V rows -> v_all
                            ps = psB.tile([128, D], F32, tag="psB")
                            for c in range(9):
                                lhs = xt_lhs(c, ts(i, 128))
                                for s2 in range(D // 512):
                                    rhs = wv[:, c, ts(s2, 512)] if c < 8 else wv_b[:, ts(s2, 512)]
                                    nc.tensor.matmul(ps[:, ts(s2, 512)], lhs, rhs,
                                                     start=(c == 0), stop=(c == 8))
                            nc.scalar.activation(v_all[:, i, :], ps[:], AF.Copy)
                            if i < NB:
                                sq = psB.tile([128, D], F32, tag="sqp", bufs=1, name="sq")
                                nc.scalar.activation(sq[:], v_all[:, i, :].bitcast(F32),
                                                     AF.Square, accum_out=norms2[:, i:i + 1])

                        # p row: sigmoid(x @ wp + b) for my columns
                        pp_ps = psP.tile([1, MY], F32)
                        for c in range(9):
                            lhs = wp_sl[:, c, :] if c < 8 else wp_b[:]
                            for s2 in range(MY // 512):
                                nc.tensor.matmul(pp_ps[:, ts(s2, 512)], lhs,
                                                 xt_lhs(c, ts(s2, 512)),
                                                 start=(c == 0), stop=(c == 8))
                        prow = sqp.tile([1, MY], F32)
                        nc.scalar.activation(prow[:], pp_ps[:], AF.Sigmoid)
                        nc.sync.dma_start(p_dram[:], prow[:])
                        nc.sync.dma_start(p_all[:], p_dram[:].rearrange("o (i p) -> p i o", p=128))

                # ============ Phases C+D (qt resident) =====================
                with tc.tile_pool(name="qt", bufs=1) as qtp:
                    qt = qtp.tile([128, 8, T], F32R)
                    nc.sync.dma_start(qt[:], qt_dram[:].rearrange("c p m -> p c m"))

                    # --- C4 first: novelty (independent of qt, covers reload)
                    with (
                        tc.tile_pool(name="c4", bufs=1) as c4p,
                        tc.tile_pool(name="c4s", bufs=2) as c4s,
                        tc.tile_pool(name="psC4", bufs=2, space="PSUM") as psC4,
                        tc.tile_pool(name="psT4", bufs=2, space="PSUM") as psT4,
                    ):
                        snt = c4p.tile([128, 8, K], F32R)
                        nc.sync.dma_start(snt[:], snT_d.rearrange("(c p) m -> p c m", p=128))
                        vtms = [c4p.tile([128, 8, 128], F32R, tag=f"vtm{i}", name=f"vtm{i}")
                                for i in range(NB)]
                        for i in range(NB):
                            for c in range(8):
                                ptr = psT4.tile([128, 128], F32R, tag="vtr", name="vtr")
                                nc.tensor.transpose(ptr[:], v_all[:, i, ts(c, 128)], identr[:])
                                nc.vector.tensor_copy(vtms[i][:, c, :], ptr[:])
                            ps = psC4.tile([128, K], F32, tag="sim", name="simps")
                            for c in range(8):
                                nc.tensor.matmul(ps[:], vtms[i][:, c, :], snt[:, c, :],
                                                 start=(c == 0), stop=(c == 7))
                            nrm = c4s.tile([128, 1], F32, tag="nrm", name="nrm")
                            nc.scalar.activation(nrm[:], norms2[:, i:i + 1], AF.Sqrt)
                            ncl = c4s.tile([128, 1], F32, tag="ncl", name="ncl")
                            nc.vector.tensor_scalar_max(ncl[:], nrm[:], EPS_COS)
                            rin = c4s.tile([128, 1], F32, tag="rin", name="rin")
                            nc.vector.reciprocal(rin[:], ncl[:])
                            sim = c4s.tile([128, K], F32, tag="sim", name="sim")
                            nc.scalar.activation(sim[:], ps[:], AF.Copy, scale=rin[:, 0:1])
                            mx = c4s.tile([128, 1], F32, tag="mx", name="mx")
                            nc.vector.tensor_reduce(mx[:], sim[:], axis=AX.X, op=ALU.max)
                            nc.scalar.activation(nv_col[:, i:i + 1], mx[:], AF.Copy,
                                                 bias=1.0, scale=-1.0)
                            ptr = psT4.tile([1, 128], F32, tag="ntr", name="ntr")
                            nc.tensor.transpose(ptr[:], nv_col[:, i:i + 1], ident[:])
                            nc.vector.tensor_copy(nvT[:, i, :], ptr[:])

                    # --- C1+C2: read logits (f32r), top-16 softmax ---------
                    with tc.tile_pool(name="rrTp", bufs=1) as rrp:
                      rrTs = [rrp.tile([128, 4, 128], F32R, tag=f"rrT{i}", name=f"rrT{i}")
                              for i in range(NB)]
                      with (
                        tc.tile_pool(name="st", bufs=1) as stp,
                        tc.tile_pool(name="c2", bufs=2) as c2p,
                        tc.tile_pool(name="psC", bufs=2, space="PSUM") as psC,
                        tc.tile_pool(name="psT", bufs=2, space="PSUM") as psTr,
                      ):
                        stile = stp.tile([128, 8, K], F32R)
                        nc.sync.dma_start(stile[:], sT_d.bitcast(F32R).rearrange("(c p) m -> p c m", p=128))
                        for i in range(NB):
                            isl = ts(i, 128)
                            ps = psC.tile([128, K], F32, tag="lg", name="lgps")
                            for c in range(8):
                                nc.tensor.matmul(ps[:], qt[:, c, isl],
                                                 stile[:, c, :], start=(c == 0), stop=(c == 7))
                            lg = c2p.tile([128, K], F32, tag="lg", name="lg")
                            nc.scalar.activation(lg[:], ps[:], AF.Copy, scale=1.0 / 32.0)
                            m1 = c2p.tile([128, 8], F32, tag="m1", name="m1")
                            m2t = c2p.tile([128, 8], F32, tag="m2", name="m2t")
                            t1 = c2p.tile([128, K], F32, tag="t1", name="t1")
                            t2 = c2p.tile([128, K], F32, tag="t2", name="t2")
                            nc.vector.max(m1[:], lg[:])
                            nc.vector.match_replace(t1[:], m1[:], lg[:], NEG)
                            nc.vector.max(m2t[:], t1[:])
                            nc.vector.match_replace(t2[:], m2t[:], t1[:], NEG)
                            nmax = c2p.tile([128, 1], F32, tag="nmax", name="nmax")
                            nc.vector.tensor_scalar_mul(nmax[:], m1[:, 0:1], -1.0)
                            e = c2p.tile([128, K], F32, tag="e", name="e")
                            nc.scalar.activation(e[:], lg[:], AF.Exp, bias=nmax[:, 0:1])
                            mk = c2p.tile([128, K], F32, tag="t1", name="mk")
                            nc.vector.tensor_scalar(mk[:], t2[:], NEG, None, ALU.is_equal)
                            den = c2p.tile([128, 1], F32, tag="den", name="den")
                            nc.vector.scalar_tensor_tensor(e[:], e[:], 1.0, mk[:],
                                                           ALU.mult, ALU.mult,
                                                           accum_out=den[:])
                            rden = c2p.tile([128, 1], F32, tag="rden", name="rden")
                            nc.vector.reciprocal(rden[:], den[:])
                            rr = c2p.tile([128, K], F32, tag="t2", name="rr")
                            nc.vector.tensor_scalar_mul(rr[:], e[:], rden[:, 0:1])
                            for kc in range(4):
                                ptr = psTr.tile([128, 128], F32, tag="tr", name="tr")
                                nc.tensor.transpose(ptr[:], rr[:, ts(kc, 128)], ident[:])
                                nc.vector.tensor_copy(rrTs[i][:, kc, :], ptr[:])
                      # --- C3: context -----------------------------------
                      with (
                        tc.tile_pool(name="c3", bufs=2) as c3p,
                        tc.tile_pool(name="psC3", bufs=2, space="PSUM") as psC3,
                      ):
                        smt = c3p.tile([128, 4, D], F32R, tag="smt", bufs=1)
                        nc.sync.dma_start(smt[:], smat_d.rearrange("(c p) m -> p c m", p=128))
                        for i in range(NB):
                            ps = psC3.tile([128, D], F32, tag="ctx", name="ctxps")
                            for kc in range(4):
                                for s2 in range(D // 512):
                                    nc.tensor.matmul(ps[:, ts(s2, 512)], rrTs[i][:, kc, :],
                                                     smt[:, kc, ts(s2, 512)],
                                                     start=(kc == 0), stop=(kc == 3))
                            cx = c3p.tile([128, D], F32, tag="cx", name="cx")
                            nc.scalar.activation(cx[:], ps[:], AF.Copy)
                            nc.sync.dma_start(ctx_dram[i], cx[:])

                    # --- C5: r_write, V_aggT, S_partT ----------------------
                    with tc.tile_pool(name="vaTp", bufs=1) as vaTp:
                      vaT = vaTp.tile([128, 8, K], F32R)
                      with (
                        tc.tile_pool(name="c5", bufs=1) as c5p,
                        tc.tile_pool(name="c5r", bufs=1) as c5rp,
                        tc.tile_pool(name="c5s", bufs=2) as c5s,
                        tc.tile_pool(name="psC5", bufs=2, space="PSUM") as psC5,
                      ):
                        wn = c5p.tile([128, 8, K], F32R)
                        wn_nov = c5p.tile([1, K], F32R)
                        wn_b = c5p.tile([1, K], F32R)
                        ones_my = c5p.tile([1, MY], F32R)  # noqa
                        nc.sync.dma_start(wn[:], wnT_d[0:D, :].rearrange("(c p) m -> p c m", p=128))
                        nc.sync.dma_start(wn_nov[:], wnT_d[D:D + 1, :])
                        nc.sync.dma_start(wn_b[:], wnT_d[D + 1:D + 2, :])
                        nc.sync.dma_start(ones_my[:], ones_d[:])
                        rww = c5rp.tile([128, 8, K], F32R)
                        for i in range(NB):
                            isl = ts(i, 128)
                            ps = psC5.tile([128, K], F32, tag="rw", name="rwps")
                            for c in range(8):
                                nc.tensor.matmul(ps[:], qt[:, c, isl], wn[:, c, :],
                                                 start=(c == 0), stop=False)
                            nc.tensor.matmul(ps[:], nvT[:, i, :], wn_nov[:],
                                             start=False, stop=False)
                            nc.tensor.matmul(ps[:], ones_my[:, isl], wn_b[:],
                                             start=False, stop=True)
                            rw = c5s.tile([128, K], F32, tag="rw", name="rw")
                            nc.scalar.activation(rw[:], ps[:], AF.Sigmoid)
                            nc.vector.tensor_scalar_mul(rww[:, i, :], rw[:],
                                                        w_col[:, i, 0:1])
                        for co in range(8):
                            ps = psC5.tile([128, K], F32, tag="va", name="vaps")
                            for i in range(NB):
                                nc.tensor.matmul(ps[:], v_all[:, i, ts(co, 128)],
                                                 rww[:, i, :],
                                                 start=(i == 0), stop=(i == NB - 1))
                            nc.vector.tensor_copy(vaT[:, co, :], ps[:])

                      # --- C5b: S_partT with streamed m2 chunks -----------
                      with (
                        tc.tile_pool(name="c5b", bufs=2) as c5b,
                        tc.tile_pool(name="c5bs", bufs=2) as c5bs,
                        tc.tile_pool(name="psC5b", bufs=8, space="PSUM") as psC5b,
                      ):
                        spps = [psC5b.tile([128, K], F32, tag="sp", name=f"spps{_co}")
                                for _co in range(8)]
                        for c in range(8):
                            m2c = c5b.tile([128, D], F32R, tag="m2c", name="m2c")
                            nc.sync.dma_start(m2c[:], m2_d[ts(c, 128), :])
                            for co in range(8):
                                nc.tensor.matmul(spps[co][:], m2c[:, ts(co, 128)],
                                                 vaT[:, c, :],
                                                 start=(c == 0), stop=(c == 7))
                        for co in range(8):
                            sp = c5bs.tile([128, K], F32, tag="sp", name="sp")
                            nc.scalar.activation(sp[:], spps[co][:], AF.Copy)
                            nc.sync.dma_start(spt_d[ts(co, 128), :], sp[:])

                    # --- Phase D: causal attention + output mix ------------
                    with (
                        tc.tile_pool(name="att", bufs=2) as attp,
                        tc.tile_pool(name="attE", bufs=1) as attep,
                        tc.tile_pool(name="msk", bufs=1) as mskp,
                        tc.tile_pool(name="aT", bufs=3) as aTp,
                        tc.tile_pool(name="byp", bufs=2) as bypp,
                        tc.tile_pool(name="ctx", bufs=2) as ctxp,
                        tc.tile_pool(name="dsm", bufs=2) as dsmp,
                        tc.tile_pool(name="psS", bufs=2, space="PSUM") as psS,
                        tc.tile_pool(name="psO", bufs=2, space="PSUM") as psO,
                        tc.tile_pool(name="psT2", bufs=2, space="PSUM") as psT2,
                    ):
                        dmask = mskp.tile([128, 128], F32)
                        lmask = mskp.tile([128, 128], F32)
                        nc.sync.dma_start(dmask[:], dmask_d[:])
                        nc.sync.dma_start(lmask[:], lmask_d[:])

                        for j in range(NB):
                            jsl = ts(j, 128)
                            kw = (j + 1) * 128
                            att = attp.tile([128, T], F32, tag="att")
                            for reg in range(2):
                                base = reg * MY
                                for sub in range(0, kw, 512):
                                    w = min(512, kw - sub)
                                    ps = psS.tile([128, 512], F32, tag="sc")
                                    for c in range(8):
                                        nc.tensor.matmul(ps[:, 0:w], qt[:, c, jsl],
                                                         qt[:, c, base + sub:base + sub + w],
                                                         start=(c == 0), stop=(c == 7))
                                    nc.scalar.activation(att[:, base + sub:base + sub + w],
                                                         ps[:, 0:w], AF.Copy, scale=1.0 / 32.0)
                            nc.vector.tensor_add(att[:, jsl], att[:, jsl], dmask[:])
                            lsl = slice(MY + j * 128, MY + kw)
                            nc.vector.tensor_add(att[:, lsl], att[:, lsl], lmask[:])
                            mx1 = dsmp.tile([128, 1], F32, tag="mx1")
                            mx2 = dsmp.tile([128, 1], F32, tag="mx2")
                            nc.vector.tensor_reduce(mx1[:], att[:, 0:kw], axis=AX.X, op=ALU.max)
                            nc.vector.tensor_reduce(mx2[:], att[:, MY:MY + kw], axis=AX.X, op=ALU.max)
                            nmx = dsmp.tile([128, 1], F32, tag="nmx")
                            nc.vector.tensor_tensor(nmx[:], mx1[:], mx2[:], op=ALU.max)
                            nc.vector.tensor_scalar_mul(nmx[:], nmx[:], -1.0)
                            atE = attep.tile([128, T], F32, tag="atE")
                            rs1 = dsmp.tile([128, 1], F32, tag="rs1")
                            rs2 = dsmp.tile([128, 1], F32, tag="rs2")
                            nc.scalar.activation(atE[:, 0:kw], att[:, 0:kw], AF.Exp,
                                                 bias=nmx[:, 0:1], accum_out=rs1[:])
                            nc.scalar.activation(atE[:, MY:MY + kw], att[:, MY:MY + kw],
                                                 AF.Exp, bias=nmx[:, 0:1], accum_out=rs2[:])
                            den = dsmp.tile([128, 1], F32, tag="den")
                            nc.vector.tensor_add(den[:], rs1[:], rs2[:])
                            rinv = dsmp.tile([128, 1], F32, tag="rinv")
                            nc.vector.reciprocal(rinv[:], den[:])
                            ob = psO.tile([128, D], F32, tag="ob")
                            nvis = 2 * (j + 1)
                            for kb in range(nvis):
                                if kb <= j:
                                    lofs, vidx = kb * 128, kb
                                else:
                                    lofs, vidx = MY + (kb - (j + 1)) * 128, 8 + (kb - (j + 1))
                                ptr = psT2.tile([128, 128], F32, tag="atr")
                                nc.tensor.transpose(ptr[:], atE[:, lofs:lofs + 128], ident[:])
                                aT = aTp.tile([128, 128], F32R, tag="aT")
                                nc.vector.tensor_copy(aT[:], ptr[:])
                                for s2 in range(2):
                                    nc.tensor.matmul(ob[:, ts(s2, 512)], aT[:],
                                                     v_all[:, vidx, ts(s2, 512)],
                                                     start=(kb == 0), stop=(kb == nvis - 1))
                            byp = bypp.tile([128, D], F32, tag="byp")
                            nc.scalar.activation(byp[:, 0:512], ob[:, 0:512], AF.Copy,
                                                 scale=rinv[:, 0:1])
                            nc.scalar.activation(byp[:, 512:D], ob[:, 512:D], AF.Copy,
                                                 scale=rinv[:, 0:1])
                            ctx = ctxp.tile([128, D], F32, tag="ctx")
                            nc.sync.dma_start(ctx[:], ctx_dram[j])
                            nc.vector.tensor_sub(byp[:], byp[:], ctx[:])
                            nc.vector.scalar_tensor_tensor(byp[:], byp[:], p_all[:, j, 0:1],
                                                           ctx[:], ALU.mult, ALU.add)
                            nc.sync.dma_start(out_d[jsl, :], byp[:])

    nc.compile()
    return nc


def _host_prep(x, S_init, Wq_w, Wq_b, Wv_w, Wv_b, Wn_w, Wn_b, Wm_w, Wm_b):
    """Build the 8 per-core input maps."""
    f32 = np.float32
    S64 = S_init.astype(np.float64)
    S_cov = S64 @ S64.T + 1e-5 * np.eye(K)
    M = S64.T @ np.linalg.inv(S_cov) @ S64
    m2 = (ETA * (np.eye(D) - M)).astype(f32)

    sn = S64 / np.maximum(np.linalg.norm(S64, axis=1, keepdims=True), EPS_COS)
    snT = np.ascontiguousarray(sn.T.astype(f32))
    sT = np.ascontiguousarray(S_init.T.astype(f32))
    smat = np.ascontiguousarray(S_init.astype(f32))

    wqT = np.concatenate([Wq_w.T, Wq_b[None, :]], axis=0).astype(f32)
    wvT = np.concatenate([Wv_w.T, Wv_b[None, :]], axis=0).astype(f32)
    wnT = np.concatenate([Wn_w[:, 1:].T, Wn_w[:, 0][None, :], Wn_b[None, :]],
                         axis=0).astype(f32)
    # p gate folded through Wq: p_logit = x@(Wq^T wm) + (bq.wm + bm)
    wm64 = Wm_w[0].astype(np.float64)
    wp = np.concatenate([Wq_w.T.astype(np.float64) @ wm64,
                         [Wq_b.astype(np.float64) @ wm64 + float(Wm_b[0])]])
    wp = np.ascontiguousarray(wp[:, None].astype(f32))

    wfull = (1.0 - LAM) ** np.arange(T - 1, -1, -1, dtype=np.float64)
    wfull = (wfull / wfull.sum()).astype(f32)

    q = np.arange(128)[:, None]
    k = np.arange(128)[None, :]
    dmask = np.where(q >= k, 0.0, NEG).astype(f32)
    ident = np.eye(128, dtype=f32)

    in_maps = []
    perms = []
    for core in range(N_CORES):
        b, h = core // 2, core % 2
        my_blocks = list(range(h, 16, 2))
        oth_blocks = list(range(1 - h, 16, 2))
        perm = np.concatenate([np.arange(g * 128, (g + 1) * 128)
                               for g in my_blocks + oth_blocks])
        perms.append(perm)
        xp = x[b][perm]  # [T, D]
        xT = np.concatenate([np.ascontiguousarray(xp.T),
                             np.ones((1, T), f32)], axis=0)
        lmask = np.full((128, 128), NEG if h == 0 else 0.0, f32)
        in_maps.append({
            "xT": np.ascontiguousarray(xT.astype(f32)),
            "wqT": wqT, "wvT": wvT, "sT": sT, "smat": smat, "snT": snT,
            "wnT": wnT, "wp": wp, "m2": m2,
            "wvec": np.ascontiguousarray(wfull[perm[:MY]][:, None]),
            "dmask": dmask, "lmask": lmask, "ident": ident,
            "ones1": np.ones((1, MY), f32),
        })
    return in_maps, perms


def _gather(results, S_init):
    f32 = np.float32
    out = np.empty((B, T, D), f32)
    S_next = np.empty((B, K, D), f32)
    for b in range(B):
        for h in range(2):
            oh = results[2 * b + h]["out_half"]
            for j in range(NB):
                g = 2 * j + h
                out[b, g * 128:(g + 1) * 128] = oh[j * 128:(j + 1) * 128]
        S_next[b] = (f32(1.0 - LAM) * S_init
                     + results[2 * b]["spartT"].T
                     + results[2 * b + 1]["spartT"].T)
    return out, S_next


def kernel(x, S_init, Wq_w, Wq_b, Wv_w, Wv_b, Wn_w, Wn_b, Wm_w, Wm_b):
    global _compiled
    from concourse import bass_utils

    x = np.asarray(x, np.float32)
    S_init = np.asarray(S_init, np.float32)
    args = [np.asarray(a, np.float32) for a in
            (Wq_w, Wq_b, Wv_w, Wv_b, Wn_w, Wn_b, Wm_w, Wm_b)]

    if _compiled is None:
        _compiled = _build_nc()
    nc = _compiled

    in_maps, _ = _host_prep(x, S_init, *args)

    trace = bool(int(os.environ.get("BASS_KERNEL_TRACE", "0")))
    if trace:
        sys.path.insert(0, os.path.dirname(os.path.abspath(__file__)))
        import ntff_shim
        ntff_shim.install()
    res = bass_utils.run_bass_kernel_spmd(
        nc, in_maps, core_ids=list(range(N_CORES)), trace=trace)
    kernel.last_result = res
    return _gather(res.results, S_init)
